# revision 1
# baseline (speedup 1.0000x reference)
"""Trainium2 Bass kernel for a 2-layer GAT block (gnn_message_passing).

Strategy (8 NeuronCores, dst-node sharding):
  - Host preprocessing: add self-loops, sort edges by dst, shard by dst range
    (6250 nodes/core), group dsts into 128-wide groups; per (core, group) pad
    the edge list to a multiple of 128; block counts are uniformized across
    cores (max over cores) so the SPMD program is identical on every core
    (only index/dstloc data differs).
  - Phase A: own-chunk node transform h1 = x @ W1ext (W1ext carries the
    attention projections a_src/a_dst and the residual Wfc as extra columns).
    Rows are written into a 320-float-per-row node table.
  - AllGather the table -> full [N, 320] table in every core's HBM.
  - Edge phase, per 128-dst group: per 128-edge block, gather the 128 src
    rows via indirect DMA (one instruction per block, 128 descriptors),
    build a one-hot selection matrix S[j, s] = (dstloc_j == s) on the vector
    engine, expand the per-dst a_dst term via PE (transpose S, then
    S_T.T @ a_dst_group), compute ex = exp(lrelu(al_src + al_dst)) on the
    scalar engine (batched per group), scale messages by ex, and segment-sum
    via matmul S.T @ [msg | ex] accumulated in PSUM over the group's blocks.
  - Layer 2 identical (plus mean over heads + residual at evacuation).
"""

import numpy as np

import concourse.bass as bass
import concourse.bacc as bacc
import concourse.mybir as mybir
import concourse.tile as tile
from concourse.bass_utils import run_bass_kernel_spmd

# Problem constants (hardcoded per harness contract)
N = 50000
E = 800000
IN_C = 128
OUT_C = 64
HEADS = 4
NEG_SLOPE = 0.2
N_CORES = 8

P = 128            # partitions
TROW = 320         # table row size in f32

FP32 = mybir.dt.float32
I32 = mybir.dt.int32

# timing-triage flags (set by timing_triage.py; default off)
SKIP_EDGE = False
SKIP_AG = False
GATHER_ONLY = False
DEBUG_DUMP = False


def _ceil_div(a, b):
    return (a + b - 1) // b


# ---------------------------------------------------------------------------
# Host-side preprocessing
# ---------------------------------------------------------------------------

def _preprocess(edge_index, n, n_cores):
    """Sort/shard/pad edges; build per-core index + dst-local arrays with a
    block schedule that is uniform across cores."""
    npc = n // n_cores
    G = _ceil_div(npc, P)

    src = np.asarray(edge_index[0], dtype=np.int64)
    dst = np.asarray(edge_index[1], dtype=np.int64)
    loops = np.arange(n, dtype=np.int64)
    src = np.concatenate([src, loops]).astype(np.int32)
    dst = np.concatenate([dst, loops]).astype(np.int32)

    order = np.argsort(dst, kind="stable")
    src = src[order]
    dst = dst[order]
    core_bounds = np.searchsorted(dst, np.arange(0, n + 1, npc))

    percore = []
    counts = np.zeros((n_cores, G), dtype=np.int64)
    for m in range(n_cores):
        s0, s1 = core_bounds[m], core_bounds[m + 1]
        cs = src[s0:s1]
        cd = dst[s0:s1] - m * npc
        grp = cd >> 7
        o = np.lexsort((cs, grp))
        cs, cd, grp = cs[o], cd[o], grp[o]
        gb = np.searchsorted(grp, np.arange(G + 1))
        counts[m] = gb[1:] - gb[:-1]
        percore.append((cs, cd, gb))

    BG = _ceil_div(counts, P).max(axis=0)  # blocks per group (uniform)
    BG = np.maximum(BG, 1)
    B_total = int(BG.sum())

    cores = []
    for m in range(n_cores):
        cs, cd, gb = percore[m]
        idx = np.zeros((P, B_total), dtype=np.int32)
        dstloc = np.full((P, B_total), -1.0, dtype=np.float32)
        t = 0
        for g in range(G):
            a, b = gb[g], gb[g + 1]
            ne = b - a
            npad = int(BG[g]) * P - ne
            e_src = np.concatenate([cs[a:b], np.zeros(npad, np.int32)])
            e_dl = np.concatenate(
                [(cd[a:b] - g * P).astype(np.float32),
                 np.full(npad, -1.0, np.float32)]
            )
            for k in range(int(BG[g])):
                idx[:, t] = e_src[k * P : (k + 1) * P]
                dstloc[:, t] = e_dl[k * P : (k + 1) * P]
                t += 1
        assert t == B_total
        cores.append(dict(idx=idx, dstloc=dstloc))

    sched = dict(G=G, npc=npc, BG=BG.astype(int), B_total=B_total,
                 MAXB=int(BG.max()))
    return sched, cores


# ---------------------------------------------------------------------------
# Device program
# ---------------------------------------------------------------------------

def _build_program(sched, n, in_c, out_c, heads, add_b1, reps=1):
    G = sched["G"]
    npc = sched["npc"]
    BG = sched["BG"]
    B_total = sched["B_total"]
    MAXB = sched["MAXB"]
    HC = heads * out_c                     # 256
    W1COLS = HC + 2 * heads + out_c        # 328: W1 | a_src | a_dst | Wfc
    W2COLS = HC + 2 * heads                # 264
    TUSED = HC + heads                     # 260 cols used of each table row

    nc = bacc.Bacc(
        "TRN2",
        target_bir_lowering=False,
        debug=False,
        enable_asserts=False,
        num_devices=N_CORES,
    )

    # ---- I/O ----
    xT = nc.dram_tensor("xT", [in_c, G * P], FP32, kind="ExternalInput")
    idx_d = nc.dram_tensor("idx", [P, B_total], I32, kind="ExternalInput")
    dstloc_d = nc.dram_tensor("dstloc", [P, B_total], FP32, kind="ExternalInput")
    w1ext_d = nc.dram_tensor("w1ext", [in_c, W1COLS], FP32, kind="ExternalInput")
    w2ext_d = nc.dram_tensor("w2ext", [HC, W2COLS], FP32, kind="ExternalInput")
    iota_d = nc.dram_tensor("iota", [P, P], FP32, kind="ExternalInput")
    ident_d = nc.dram_tensor("ident", [P, P], FP32, kind="ExternalInput")
    if add_b1:
        b1rep_d = nc.dram_tensor("b1rep", [P, HC], FP32, kind="ExternalInput")
    out_d = nc.dram_tensor("out", [G * P, out_c], FP32, kind="ExternalOutput")
    if DEBUG_DUMP:
        dbg_f1 = nc.dram_tensor("dbg_f1", [P, G * HC], FP32, kind="ExternalOutput")
        dbg_t1 = nc.dram_tensor("dbg_t1", [512, TROW], FP32, kind="ExternalOutput")
        dbg_t2 = nc.dram_tensor("dbg_t2", [512, TROW], FP32, kind="ExternalOutput")
        dbg_al = nc.dram_tensor("dbg_al", [P, G * heads], FP32, kind="ExternalOutput")
        dbg_al2 = nc.dram_tensor("dbg_al2", [P, G * heads], FP32, kind="ExternalOutput")
        dbg_hsum = nc.dram_tensor("dbg_hsum", [P, G * out_c], FP32, kind="ExternalOutput")
        dbg_xch = nc.dram_tensor("dbg_xch", [P, G * out_c], FP32, kind="ExternalOutput")

    with tile.TileContext(nc) as tc:
        with (
            tc.tile_pool(name="const", bufs=1) as cpool,
            tc.tile_pool(name="dram", bufs=1, space="DRAM") as dpool,
        ):
            iota_t = cpool.tile([P, P], FP32)
            nc.sync.dma_start(out=iota_t[:], in_=iota_d[:])
            ident_t = cpool.tile([P, P], FP32)
            nc.sync.dma_start(out=ident_t[:], in_=ident_d[:])
            w1_t = cpool.tile([in_c, W1COLS], FP32)
            nc.sync.dma_start(out=w1_t[:], in_=w1ext_d[:])
            w2a_t = cpool.tile([P, W2COLS], FP32)
            nc.sync.dma_start(out=w2a_t[:], in_=w2ext_d[0:P, :])
            w2b_t = cpool.tile([P, W2COLS], FP32)
            nc.sync.dma_start(out=w2b_t[:], in_=w2ext_d[P : 2 * P, :])
            if add_b1:
                b1_t = cpool.tile([P, HC], FP32)
                nc.sync.dma_start(out=b1_t[:], in_=b1rep_d[:])

            idx_t = cpool.tile([P, B_total], I32)
            nc.sync.dma_start(out=idx_t[:], in_=idx_d[:])
            dstloc_t = cpool.tile([P, B_total], FP32)
            nc.sync.dma_start(out=dstloc_t[:], in_=dstloc_d[:])

            f1_sb = cpool.tile([P, G * HC], FP32)
            xch_sb = cpool.tile([P, G * out_c], FP32)
            alde1_sb = cpool.tile([P, G * heads], FP32)
            alde2_sb = cpool.tile([P, G * heads], FP32)

            table1_own = dpool.tile([npc, TROW], FP32)
            table1 = dpool.tile([n, TROW], FP32, addr_space="Shared")
            table2_own = dpool.tile([npc, TROW], FP32)
            table2 = dpool.tile([n, TROW], FP32, addr_space="Shared")

            for _rep in range(reps):
              table1_own = dpool.tile([npc, TROW], FP32, tag=f"t1o{_rep}",
                                      name=f"table1_own{_rep}")
              table1 = dpool.tile([n, TROW], FP32, addr_space="Shared",
                                  tag=f"t1{_rep}", name=f"table1{_rep}")
              table2_own = dpool.tile([npc, TROW], FP32, tag=f"t2o{_rep}",
                                      name=f"table2_own{_rep}")
              table2 = dpool.tile([n, TROW], FP32, addr_space="Shared",
                                  tag=f"t2{_rep}", name=f"table2{_rep}")
              # ---------------- Phase A: layer-1 node transform ----------------
              with (
                  tc.tile_pool(name=f"pa{_rep}", bufs=3) as pa,
                  tc.tile_pool(name=f"pa_ps{_rep}", bufs=2, space="PSUM") as pa_ps,
              ):
                  for g in range(G):
                      xt_t = pa.tile([in_c, P], FP32, tag="xt")
                      nc.sync.dma_start(out=xt_t[:], in_=xT[:, g * P : (g + 1) * P])
                      ph = pa_ps.tile([P, W1COLS], FP32, tag="ph")
                      nc.tensor.matmul(
                          ph[:], lhsT=xt_t[:], rhs=w1_t[:], start=True, stop=True
                      )
                      rows = min(P, npc - g * P)
                      tx = pa.tile([P, TUSED], FP32, tag="tx")
                      nc.scalar.copy(tx[:], ph[:, 0:TUSED])
                      nc.vector.tensor_copy(
                          alde1_sb[:, g * heads : (g + 1) * heads],
                          ph[:, TUSED : TUSED + heads],
                      )
                      nc.vector.tensor_copy(
                          xch_sb[:, g * out_c : (g + 1) * out_c],
                          ph[:, HC + 2 * heads : W1COLS],
                      )
                      nc.sync.dma_start(
                          out=table1_own[g * P : g * P + rows, 0:TUSED],
                          in_=tx[:rows, :],
                      )

              if not SKIP_AG:
                  nc.gpsimd.collective_compute(
                      "AllGather",
                      mybir.AluOpType.bypass,
                      replica_groups=[list(range(N_CORES))],
                      ins=[table1_own[:].opt()],
                      outs=[table1[:].opt()],
                  )

              # ---------------- Edge phase ----------------
              def edge_phase(table, alde_sb, layer):
                  with (
                      tc.tile_pool(name=f"gt{layer}", bufs=2) as gpool,
                      tc.tile_pool(name=f"sS{layer}", bufs=MAXB + 2) as spool,
                      tc.tile_pool(name=f"sT{layer}", bufs=3) as stpool,
                      tc.tile_pool(name=f"ev{layer}", bufs=3) as evpool,
                      tc.tile_pool(name=f"pse{layer}", bufs=2, space="PSUM") as pse,
                      tc.tile_pool(name=f"pst{layer}", bufs=2, space="PSUM") as pst,
                      tc.tile_pool(name=f"pso{layer}", bufs=2, space="PSUM") as pso,
                  ):
                      t = 0
                      for g in range(G):
                          nblk = int(BG[g])
                          gt = gpool.tile([P, MAXB * TROW], FP32, tag="g")
                          gt3 = gt[:].rearrange("p (c e) -> p c e", e=TROW)
                          for i in range(nblk):
                              nc.gpsimd.indirect_dma_start(
                                  out=gt3[:, i, :],
                                  out_offset=None,
                                  in_=table[:],
                                  in_offset=bass.IndirectOffsetOnAxis(
                                      ap=idx_t[:, t + i : t + i + 1], axis=0
                                  ),
                              )

                          if GATHER_ONLY:
                              t += nblk
                              continue
                          ps_e = pse.tile([P, MAXB * heads], FP32, tag="pe")
                          s_tiles = []
                          for i in range(nblk):
                              S = spool.tile([P, P], FP32, tag="S")
                              nc.vector.tensor_scalar(
                                  S[:],
                                  iota_t[:],
                                  dstloc_t[:, t + i : t + i + 1],
                                  None,
                                  mybir.AluOpType.is_equal,
                              )
                              s_tiles.append(S)
                              pSt = pst.tile([P, P], FP32, tag="pst")
                              nc.tensor.transpose(pSt[:], S[:], ident_t[:])
                              St = stpool.tile([P, P], FP32, tag="St")
                              nc.scalar.copy(St[:], pSt[:])
                              nc.tensor.matmul(
                                  ps_e[:, i * heads : (i + 1) * heads],
                                  lhsT=St[:],
                                  rhs=alde_sb[:, g * heads : (g + 1) * heads],
                                  start=True,
                                  stop=True,
                              )

                          # ex = exp(lrelu(al_src + al_dst)), batched per group,
                          # written into table-row padding (cols 260:264)
                          exv = bass.AP(
                              gt3.tensor,
                              gt3.offset + TUSED,
                              [gt3.ap[0], [TROW, nblk], [1, heads]],
                          )
                          alsrc = bass.AP(
                              gt3.tensor,
                              gt3.offset + HC,
                              [gt3.ap[0], [TROW, nblk], [1, heads]],
                          )
                          nc.vector.tensor_tensor(
                              out=exv,
                              in0=ps_e[:, 0 : nblk * heads].rearrange(
                                  "p (c e) -> p c e", e=heads
                              ),
                              in1=alsrc,
                              op=mybir.AluOpType.add,
                          )
                          # lrelu(z) = max(z, 0.2*z) computed manually (the HW
                          # Lrelu table has a fixed slope, ignoring alpha)
                          lrt = evpool.tile([P, MAXB * heads], FP32, tag="lrt")
                          lrt3 = lrt[:].rearrange("p (c e) -> p c e", e=heads)[
                              :, 0:nblk, :
                          ]
                          nc.vector.tensor_scalar(
                              lrt3, exv, NEG_SLOPE, None, mybir.AluOpType.mult
                          )
                          nc.vector.tensor_tensor(
                              out=exv, in0=exv, in1=lrt3, op=mybir.AluOpType.max
                          )
                          nc.scalar.activation(
                              exv, exv, mybir.ActivationFunctionType.Exp
                          )

                          ps_out = pso.tile([P, TUSED + heads], FP32, tag="po")
                          for i in range(nblk):
                              msg = gt3[:, i, 0:HC]
                              exs = bass.AP(
                                  gt3.tensor,
                                  gt3.offset + (i * TROW + TUSED),
                                  [gt3.ap[0], [1, heads], [0, out_c]],
                              )
                              nc.vector.tensor_tensor(
                                  out=msg, in0=msg, in1=exs,
                                  op=mybir.AluOpType.mult,
                              )
                              nc.tensor.matmul(
                                  ps_out[:],
                                  lhsT=s_tiles[i][:],
                                  rhs=gt3[:, i, 0 : TUSED + heads],
                                  start=(i == 0),
                                  stop=(i == nblk - 1),
                              )
                          t += nblk

                          # ---- evacuate group ----
                          rec = evpool.tile([P, heads], FP32, tag="rec")
                          if str(layer).endswith("_1"):
                              nc.vector.tensor_scalar(
                                  rec[:], ps_out[:, TUSED : TUSED + heads],
                                  1e-16, None, mybir.AluOpType.add,
                              )
                              nc.vector.reciprocal(rec[:], rec[:])
                              recb = bass.AP(
                                  rec[:].tensor, rec[:].offset,
                                  [rec[:].ap[0], [1, heads], [0, out_c]],
                              )
                              nc.vector.tensor_tensor(
                                  out=f1_sb[:, g * HC : (g + 1) * HC],
                                  in0=ps_out[:, 0:HC],
                                  in1=recb,
                                  op=mybir.AluOpType.mult,
                              )
                              if add_b1:
                                  nc.vector.tensor_tensor(
                                      out=f1_sb[:, g * HC : (g + 1) * HC],
                                      in0=f1_sb[:, g * HC : (g + 1) * HC],
                                      in1=b1_t[:],
                                      op=mybir.AluOpType.add,
                                  )
                          else:
                              nc.vector.tensor_scalar(
                                  rec[:], ps_out[:, TUSED : TUSED + heads],
                                  1e-16, float(heads),
                                  mybir.AluOpType.add, mybir.AluOpType.mult,
                              )
                              nc.vector.reciprocal(rec[:], rec[:])
                              recb = bass.AP(
                                  rec[:].tensor, rec[:].offset,
                                  [rec[:].ap[0], [1, heads], [0, out_c]],
                              )
                              tmp = evpool.tile([P, HC], FP32, tag="tmp")
                              nc.vector.tensor_tensor(
                                  out=tmp[:], in0=ps_out[:, 0:HC], in1=recb,
                                  op=mybir.AluOpType.mult,
                              )
                              hsum = evpool.tile([P, out_c], FP32, tag="hsum")
                              tmpv = bass.AP(
                                  tmp[:].tensor, tmp[:].offset,
                                  [tmp[:].ap[0], [1, out_c], [out_c, heads]],
                              )
                              nc.vector.tensor_reduce(
                                  out=hsum[:], in_=tmpv,
                                  axis=mybir.AxisListType.X,
                                  op=mybir.AluOpType.add,
                              )
                              if DEBUG_DUMP:
                                  nc.sync.dma_start(
                                      out=dbg_hsum[:, g * out_c : (g + 1) * out_c],
                                      in_=hsum[:],
                                  )
                              ob = evpool.tile([P, out_c], FP32, tag="ob")
                              nc.vector.tensor_tensor(
                                  out=ob[:], in0=hsum[:],
                                  in1=xch_sb[:, g * out_c : (g + 1) * out_c],
                                  op=mybir.AluOpType.add,
                              )
                              nc.sync.dma_start(
                                  out=out_d[g * P : (g + 1) * P, :], in_=ob[:]
                              )

              if SKIP_EDGE or GATHER_ONLY:
                  nc.vector.memset(f1_sb[:], 0.0)
              if SKIP_EDGE:
                  pass
              else:
                  edge_phase(table1, alde1_sb, layer=f"{_rep}_1")

              # ---------------- Phase D: layer-2 node transform ----------------
              with (
                  tc.tile_pool(name=f"pd{_rep}", bufs=3) as pd,
                  tc.tile_pool(name=f"pd_ps{_rep}", bufs=2, space="PSUM") as pd_ps,
                  tc.tile_pool(name=f"pd_pt{_rep}", bufs=2, space="PSUM") as pd_pt,
              ):
                  for g in range(G):
                      ph = pd_ps.tile([P, W2COLS], FP32, tag="ph2")
                      for k in range(2):
                          pft = pd_pt.tile([P, P], FP32, tag="pft")
                          nc.tensor.transpose(
                              pft[:],
                              f1_sb[:, g * HC + k * P : g * HC + (k + 1) * P],
                              ident_t[:],
                          )
                          fT = pd.tile([P, P], FP32, tag="fT")
                          nc.scalar.copy(fT[:], pft[:])
                          nc.tensor.matmul(
                              ph[:],
                              lhsT=fT[:],
                              rhs=(w2a_t if k == 0 else w2b_t)[:],
                              start=(k == 0),
                              stop=(k == 1),
                          )
                      rows = min(P, npc - g * P)
                      tx = pd.tile([P, TUSED], FP32, tag="tx2")
                      nc.scalar.copy(tx[:], ph[:, 0:TUSED])
                      nc.vector.tensor_copy(
                          alde2_sb[:, g * heads : (g + 1) * heads],
                          ph[:, TUSED : TUSED + heads],
                      )
                      nc.sync.dma_start(
                          out=table2_own[g * P : g * P + rows, 0:TUSED],
                          in_=tx[:rows, :],
                      )

              if not SKIP_AG:
                  nc.gpsimd.collective_compute(
                      "AllGather",
                      mybir.AluOpType.bypass,
                      replica_groups=[list(range(N_CORES))],
                      ins=[table2_own[:].opt()],
                      outs=[table2[:].opt()],
                  )

              if DEBUG_DUMP:
                  nc.sync.dma_start(out=dbg_f1[:], in_=f1_sb[:])
                  nc.sync.dma_start(out=dbg_al[:], in_=alde1_sb[:])
                  nc.sync.dma_start(out=dbg_al2[:], in_=alde2_sb[:])
                  nc.sync.dma_start(out=dbg_xch[:], in_=xch_sb[:])
                  with tc.tile_pool(name=f"dbgp{_rep}", bufs=2) as dbgp:
                      for q in range(4):
                          dt_ = dbgp.tile([P, TROW], FP32, tag="dbg")
                          nc.sync.dma_start(
                              out=dt_[:],
                              in_=table1[20000 + q * P : 20000 + (q + 1) * P, :],
                          )
                          nc.sync.dma_start(
                              out=dbg_t1[q * P : (q + 1) * P, :], in_=dt_[:]
                          )
                      for q in range(4):
                          dt2_ = dbgp.tile([P, TROW], FP32, tag="dbg2")
                          nc.sync.dma_start(
                              out=dt2_[:],
                              in_=table2[20000 + q * P : 20000 + (q + 1) * P, :],
                          )
                          nc.sync.dma_start(
                              out=dbg_t2[q * P : (q + 1) * P, :], in_=dt2_[:]
                          )
              if not SKIP_EDGE:
                  edge_phase(table2, alde2_sb, layer=f"{_rep}_2")
              elif True:
                  ob0 = cpool.tile([P, out_c], FP32)
                  nc.vector.memset(ob0[:], 0.0)
                  for g in range(G):
                      nc.sync.dma_start(out=out_d[g * P : (g + 1) * P, :], in_=ob0[:])

    nc.compile()
    return nc


# ---------------------------------------------------------------------------
# Entry point
# ---------------------------------------------------------------------------

def kernel(x, edge_index, W1, a_src1, a_dst1, b1, W2, a_src2, a_dst2, b2,
           Wfc, bfc):
    x = np.asarray(x, dtype=np.float32)
    W1 = np.asarray(W1, dtype=np.float32)
    W2 = np.asarray(W2, dtype=np.float32)
    a_src1 = np.asarray(a_src1, dtype=np.float32)
    a_dst1 = np.asarray(a_dst1, dtype=np.float32)
    a_src2 = np.asarray(a_src2, dtype=np.float32)
    a_dst2 = np.asarray(a_dst2, dtype=np.float32)
    Wfc = np.asarray(Wfc, dtype=np.float32)
    b1 = np.asarray(b1, dtype=np.float32)
    b2 = np.asarray(b2, dtype=np.float32)
    bfc = np.asarray(bfc, dtype=np.float32)

    n, in_c = x.shape
    heads, out_c = a_src1.shape
    hc = heads * out_c
    add_b1 = bool(np.any(b1 != 0))

    sched, cores = _preprocess(edge_index, n, N_CORES)
    G, npc = sched["G"], sched["npc"]

    w1r = W1.reshape(in_c, heads, out_c)
    w1_as = np.einsum("khc,hc->kh", w1r, a_src1)
    w1_ad = np.einsum("khc,hc->kh", w1r, a_dst1)
    w1ext = np.concatenate([W1, w1_as, w1_ad, Wfc], axis=1).astype(np.float32)
    w2r = W2.reshape(hc, heads, out_c)
    w2_as = np.einsum("khc,hc->kh", w2r, a_src2)
    w2_ad = np.einsum("khc,hc->kh", w2r, a_dst2)
    w2ext = np.concatenate([W2, w2_as, w2_ad], axis=1).astype(np.float32)

    iota = np.broadcast_to(np.arange(P, dtype=np.float32), (P, P)).copy()
    ident = np.eye(P, dtype=np.float32)

    nc = _build_program(sched, n, in_c, out_c, heads, add_b1)

    in_maps = []
    for m in range(N_CORES):
        xpad = np.zeros((G * P, in_c), dtype=np.float32)
        xpad[:npc] = x[m * npc : (m + 1) * npc]
        im = dict(
            xT=np.ascontiguousarray(xpad.T),
            idx=cores[m]["idx"],
            dstloc=cores[m]["dstloc"],
            w1ext=w1ext,
            w2ext=w2ext,
            iota=iota,
            ident=ident,
        )
        if add_b1:
            im["b1rep"] = np.broadcast_to(b1, (P, hc)).copy()
        in_maps.append(im)

    res = run_bass_kernel_spmd(nc, in_maps, list(range(N_CORES)))
    global LAST_RESULTS
    LAST_RESULTS = res
    outs = [res.results[m]["out"][:npc] for m in range(N_CORES)]
    out = np.concatenate(outs, axis=0)
    out = out + (b2 + bfc)[None, :].astype(np.float32)
    return out.astype(np.float32)

